# revision 1
# baseline (speedup 1.0000x reference)
"""Trainium2 Bass kernel for nn_Attention_45303315038988.

  q = p @ Wh.T (+bh) ; k = r @ Wl.T + bl ; v = p @ Wg.T + bg     [N, D]
  scores = q @ k.T ; attn = softmax(scores, axis=0) ; out = p + attn @ v

Design (8 NeuronCores, sequence-parallel over the query/row axis):
  - Host pre-transposes shards to feature-major fp16 (pT, rT, W^T) so every
    matmul contracts over the SBUF partition axis; no on-device transposes.
    bh is dropped: it only shifts scores by a per-key constant, which the
    softmax over the query axis cancels exactly.
  - Each core computes its shard of k^T ([d, j], bias via per-partition ACT
    bias) and v ([j, d], bias via a K=1 ones-row matmul); both AllGathered
    in fp16. AG(k^T) is kicked first, and the first K^T block + first V
    tiles are prefetched on the gpsimd queue ahead of later collectives so
    the PE never waits on queue-ordering.
  - scores^T = K^T.T @ q^T puts the softmax axis on the free dim: per-key
    max is a DVE reduction, E = exp(s - m_local) goes fp16-resident in SBUF
    (16 MB), per-key sums are DVE reductions over E.
  - Softmax globalization: (max,sum) stats are AllGathered in two halves
    (the first hides under phase C) and combined locally; the correction
    exp(m_local - M)/S is folded into E rows by ACT per-partition scales.
  - out = E^T.T @ V over 64 key blocks in two i-half passes (4 PSUM
    accumulators x 2 banks each); each E-block weight load feeds 2 matmuls;
    V streams fp16 with contiguous 2 KB rows; residual p added in fp32.
All matmul operands are fp16 with fp32 PSUM accumulation; softmax
statistics are fp32. Measured rel-to-absmax error ~2.4e-3 (fp32 reference).
Measured HW time: ~0.67-0.75 ms/run (8-core TRN2, wall-clock-differenced).
"""
import numpy as np

P = 128
D = 1024
N = 8192
NCORES = 8
NL = N // NCORES
DB = D // P
JBL = NL // P
NG = N // P
IB = NL // P
FH = 512


def build_nc(k_iters: int = 1, no_cc: bool = False, phases: str = "full",
             opts: dict | None = None, spin_us: int = 0):
    opts = opts or {}
    import concourse.mybir as mybir
    import concourse.tile as tile
    from concourse import bacc

    f16 = mybir.dt.float16
    f32 = mybir.dt.float32
    AF = mybir.ActivationFunctionType
    AX = mybir.AxisListType
    ALU = mybir.AluOpType
    RG = [list(range(NCORES))]

    nc = bacc.Bacc("TRN2", target_bir_lowering=False, debug=False,
                   num_devices=1 if no_cc else NCORES)

    def collective(kind, op, ins, outs):
        if no_cc:
            src_ap, dst_ap = ins[0], outs[0]
            nc.sync.dma_start(out=dst_ap[0] if kind == "AllGather" else dst_ap[:],
                              in_=src_ap[:])
        else:
            nc.gpsimd.collective_compute(kind, op, replica_groups=RG,
                                         ins=[ins[0].opt()], outs=[outs[0].opt()])

    pT_h = nc.dram_tensor("pT", [D, NL], f16, kind="ExternalInput")
    rT_h = nc.dram_tensor("rT", [D, NL], f16, kind="ExternalInput")
    pres_h = nc.dram_tensor("pres", [NL, D], f32, kind="ExternalInput")
    WhT_h = nc.dram_tensor("WhT", [D, D], f16, kind="ExternalInput")
    WlT_h = nc.dram_tensor("WlT", [D, D], f16, kind="ExternalInput")
    WgT_h = nc.dram_tensor("WgT", [D, D], f16, kind="ExternalInput")
    bl_h = nc.dram_tensor("bl_r", [P, DB], f32, kind="ExternalInput")
    bg_h = nc.dram_tensor("bg16", [1, D], f16, kind="ExternalInput")
    ones_h = nc.dram_tensor("ones16", [1, P], f16, kind="ExternalInput")
    out_h = nc.dram_tensor("out", [NL, D], f32, kind="ExternalOutput")

    with tile.TileContext(nc) as tc:
        with tc.tile_pool(name="dram", bufs=1, space="DRAM") as dpool:
            for it in range(k_iters):
                cc_kt_in = dpool.tile([D, NL], f16, name=f"cc_kt_in{it}")
                cc_kt_out = dpool.tile([NCORES, D, NL], f16,
                                       addr_space="Shared", name=f"cc_kt_out{it}")
                cc_v_in = dpool.tile([NL, D], f16, name=f"cc_v_in{it}")
                cc_v_out = dpool.tile([NCORES, NL, D], f16,
                                      addr_space="Shared", name=f"cc_v_out{it}")
                cc_st_in = [dpool.tile([P, NG], f32, name=f"cc_st_in{it}_{h}")
                            for h in range(2)]
                cc_st_out = [dpool.tile([NCORES, P, NG], f32, addr_space="Shared",
                                        name=f"cc_st_out{it}_{h}")
                             for h in range(2)]

                with tc.tile_pool(name="lp", bufs=1) as lp:
                    qT = lp.tile([P, DB, NL], f16)
                    stats = lp.tile([P, 2, NG], f32)   # [:,0,:]=-max, [:,1,:]=sum
                    f_sc = lp.tile([P, NG], f32)
                    bl_sb = lp.tile([P, DB], f32)
                    bg_sb = lp.tile([1, D], f16)
                    ones_sb = lp.tile([1, P], f16)
                    nc.sync.dma_start(out=bl_sb, in_=bl_h.ap())
                    nc.sync.dma_start(out=bg_sb, in_=bg_h.ap())
                    nc.sync.dma_start(out=ones_sb, in_=ones_h.ap())

                    if phases.startswith("C"):
                        # scores-loop microbench: fake qT/kt from inputs
                        ep_cm = tc.tile_pool(name="ep", bufs=1)
                        ep = ep_cm.__enter__()
                        E = ep.tile([P, NG, NL], f16)
                        with (
                            tc.tile_pool(name="ktp", bufs=2) as ktp,
                            tc.tile_pool(name="psC", bufs=3, space="PSUM") as psC,
                        ):
                            for db in range(DB):
                                nc.gpsimd.dma_start(
                                    out=qT[:, db, :],
                                    in_=pT_h.ap()[db * P:(db + 1) * P, :])
                            for c_idx in range(NCORES):
                                kt_c = ktp.tile([P, DB, JBL, P], f16, tag="kt")
                                for db in range(DB):
                                    nc.gpsimd.dma_start(
                                        out=kt_c[:, db, :, :].rearrange(
                                            "p a b -> p (a b)"),
                                        in_=WlT_h.ap()[db * P:(db + 1) * P, :])
                                for jlb in range(JBL):
                                    g = c_idx * JBL + jlb
                                    ps_t = psC.tile([P, NL], f32, tag="sc")
                                    for db in range(DB):
                                        for ih in range(2):
                                            nc.tensor.matmul(
                                                ps_t[:, ih * FH:(ih + 1) * FH],
                                                lhsT=kt_c[:, 0, 0, :] if phases == "C6"
                                                else kt_c[:, db, jlb, :],
                                                rhs=qT[:, db, ih * FH:(ih + 1) * FH],
                                                start=(db == 0), stop=(db == DB - 1))
                                    if phases in ("C2", "C3", "C4"):
                                        nc.vector.tensor_reduce(
                                            out=stats[:, 0, g:g + 1], in_=ps_t,
                                            op=ALU.max, axis=AX.X, negate=True)
                                    if phases == "C3":
                                        nc.scalar.activation(
                                            out=E[:, g, :], in_=ps_t, func=AF.Exp,
                                            bias=stats[:, 0, g:g + 1], scale=1.0,
                                            accum_out=stats[:, 1, g:g + 1])
                                    if phases == "C4":
                                        nc.scalar.activation(
                                            out=E[:, g, :], in_=ps_t, func=AF.Exp,
                                            bias=stats[:, 0, g:g + 1], scale=1.0)
                                    if phases == "C5":
                                        nc.scalar.activation(
                                            out=E[:, g, :], in_=ps_t, func=AF.Exp)
                            if phases in ("C1", "C6"):
                                pe_t = ktp.tile([P, NL], f32, tag="pe2")
                                nc.vector.tensor_copy(out=pe_t, in_=ps_t)
                                nc.sync.dma_start(out=out_h.ap()[0:P, :], in_=pe_t)
                            elif phases == "C3":
                                nc.sync.dma_start(out=out_h.ap()[0:P, 0:2 * NG],
                                    in_=stats.rearrange("p a b -> p (a b)"))
                            elif phases == "C4":
                                nc.sync.dma_start(out=out_h.ap()[0:P, 0:NG],
                                    in_=stats[:, 0, :])
                            else:
                                pe_t = ktp.tile([P, NL], f32, tag="pe2")
                                nc.vector.tensor_copy(out=pe_t, in_=E[:, NG - 1, :])
                                nc.sync.dma_start(out=out_h.ap()[0:P, :], in_=pe_t)
                        ep_cm.__exit__(None, None, None)
                        continue

                    # ---------------- phase A: projections ----------------
                    kt_pre = lp.tile([P, DB, JBL, P], f16, name="kt_pre")
                    with (
                        tc.tile_pool(name="pw", bufs=1) as pw,
                        tc.tile_pool(name="pst", bufs=3) as pst,
                        tc.tile_pool(name="psA", bufs=2, space="PSUM") as psA,
                    ):
                        WhT_sb = pw.tile([P, DB, D], f16)
                        WlT_sb = pw.tile([P, DB, D], f16)
                        WgT_sb = pw.tile([P, DB, D], f16)
                        pT_sb = pw.tile([P, DB, NL], f16)
                        rT_sb = pw.tile([P, DB, NL], f16)
                        # per-db loads, interleaved so the first projection's
                        # first matmul (needs WlT[db0] + rT[db0]) starts asap
                        ldeng = nc.sync if opts.get("hwdge_loads") else nc.gpsimd
                        for db in range(DB):
                            for t_sb, t_h in ((WlT_sb, WlT_h), (rT_sb, rT_h)):
                                ldeng.dma_start(
                                    out=t_sb[:, db, :],
                                    in_=t_h.ap()[db * P:(db + 1) * P, :])
                        for t_sb, t_h in ((WgT_sb, WgT_h), (pT_sb, pT_h),
                                          (WhT_sb, WhT_h)):
                            for db in range(DB):
                                ldeng.dma_start(
                                    out=t_sb[:, db, :],
                                    in_=t_h.ap()[db * P:(db + 1) * P, :])

                        # k^T shard = Wl^T.T @ r^T -> [do, j_l], + bl
                        for dob in range(DB):
                            ps_t = psA.tile([P, NL], f32)
                            for db in range(DB):
                                for ih in range(2):
                                    nc.tensor.matmul(
                                        ps_t[:, ih * FH:(ih + 1) * FH],
                                        lhsT=WlT_sb[:, db, dob * P:(dob + 1) * P],
                                        rhs=rT_sb[:, db, ih * FH:(ih + 1) * FH],
                                        start=(db == 0), stop=(db == DB - 1))
                            st = pst.tile([P, NL], f16, tag="st")
                            nc.scalar.activation(out=st, in_=ps_t, func=AF.Identity,
                                                 bias=bl_sb[:, dob:dob + 1], scale=1.0)
                            nc.sync.dma_start(out=cc_kt_in[dob * P:(dob + 1) * P, :],
                                              in_=st)
                        collective("AllGather", ALU.bypass, [cc_kt_in], [cc_kt_out])
                        # prefetch first K^T block ahead of AG(v) on gpsimd
                        for db in range(DB):
                            nc.gpsimd.dma_start(
                                out=kt_pre[:, db, :, :].rearrange("p a b -> p (a b)"),
                                in_=cc_kt_out[0, db * P:(db + 1) * P, :])

                        # v shard = p^T.T @ Wg^T -> [j_l, dv], + bg via ones-row
                        for jb in range(JBL):
                            ps_t = psA.tile([P, NL], f32)
                            for db in range(DB):
                                for dvh in range(2):
                                    nc.tensor.matmul(
                                        ps_t[:, dvh * FH:(dvh + 1) * FH],
                                        lhsT=pT_sb[:, db, jb * P:(jb + 1) * P],
                                        rhs=WgT_sb[:, db, dvh * FH:(dvh + 1) * FH],
                                        start=(db == 0), stop=False)
                            for dvh in range(2):
                                nc.tensor.matmul(
                                    ps_t[:, dvh * FH:(dvh + 1) * FH],
                                    lhsT=ones_sb[:, :],
                                    rhs=bg_sb[:, dvh * FH:(dvh + 1) * FH],
                                    start=False, stop=True)
                            st = pst.tile([P, NL], f16, tag="st")
                            nc.scalar.activation(out=st, in_=ps_t, func=AF.Copy)
                            nc.sync.dma_start(out=cc_v_in[jb * P:(jb + 1) * P, :],
                                              in_=st)
                        collective("AllGather", ALU.bypass, [cc_v_in], [cc_v_out])

                        # q^T = Wh^T.T @ p^T -> [do, i], + bh; stays in SBUF
                        for dob in range(DB):
                            ps_t = psA.tile([P, NL], f32)
                            for db in range(DB):
                                for ih in range(2):
                                    nc.tensor.matmul(
                                        ps_t[:, ih * FH:(ih + 1) * FH],
                                        lhsT=WhT_sb[:, db, dob * P:(dob + 1) * P],
                                        rhs=pT_sb[:, db, ih * FH:(ih + 1) * FH],
                                        start=(db == 0), stop=(db == DB - 1))
                            nc.scalar.activation(out=qT[:, dob, :], in_=ps_t,
                                                 func=AF.Copy)

                    if phases == "A":
                        with tc.tile_pool(name="probe", bufs=2) as prb:
                            pe_t = prb.tile([P, NL], f32, tag="pe")
                            nc.vector.tensor_copy(out=pe_t, in_=qT[:, 0, :])
                            nc.sync.dma_start(out=out_h.ap()[0:P, :], in_=pe_t)
                        continue

                    # -------- phase C: scores^T + local stats --------
                    ep_cm = tc.tile_pool(name="ep", bufs=1)
                    ep = ep_cm.__enter__()
                    E = ep.tile([P, NG, NL], f16)
                    with (
                        tc.tile_pool(name="ktp", bufs=2) as ktp,
                        tc.tile_pool(name="psC", bufs=3, space="PSUM") as psC,
                    ):
                        for c_idx in range(NCORES):
                            if c_idx == 0:
                                kt_c = kt_pre
                            else:
                                kt_c = ktp.tile([P, DB, JBL, P], f16, tag="kt")
                                for db in range(DB):
                                    nc.gpsimd.dma_start(
                                        out=kt_c[:, db, :, :].rearrange(
                                            "p a b -> p (a b)"),
                                        in_=cc_kt_out[c_idx,
                                                      db * P:(db + 1) * P, :])
                            for jlb in range(JBL):
                                g = c_idx * JBL + jlb
                                ps_t = psC.tile([P, NL], f32, tag="sc")
                                for db in range(DB):
                                    for ih in range(2):
                                        nc.tensor.matmul(
                                            ps_t[:, ih * FH:(ih + 1) * FH],
                                            lhsT=kt_c[:, db, jlb, :],
                                            rhs=qT[:, db, ih * FH:(ih + 1) * FH],
                                            start=(db == 0), stop=(db == DB - 1))
                                nc.vector.tensor_reduce(
                                    out=stats[:, 0, g:g + 1], in_=ps_t,
                                    op=ALU.max, axis=AX.X, negate=True)
                                nc.scalar.activation(
                                    out=E[:, g, :], in_=ps_t, func=AF.Exp,
                                    bias=stats[:, 0, g:g + 1], scale=1.0)
                                nc.vector.tensor_reduce(
                                    out=stats[:, 1, g:g + 1], in_=E[:, g, :],
                                    op=ALU.add, axis=AX.X)

                    # prefetch the first phase-E V tiles before the stats
                    # collectives block the gpsimd queue
                    vtp_cm = tc.tile_pool(name="vtp", bufs=6)
                    vtp = vtp_cm.__enter__()
                    vt_pre = []
                    for g in range(4):
                        c_idx, jlb = divmod(g, JBL)
                        vt = vtp.tile([P, D], f16, tag="vt", name=f"vtpre{g}")
                        nc.gpsimd.dma_start(
                            out=vt, in_=cc_v_out[c_idx, jlb * P:(jlb + 1) * P, :])
                        vt_pre.append(vt)

                    with (
                        tc.tile_pool(name="ktp2", bufs=1) as _unused_ktp2,
                    ):
                        # stats AllGather + combine in two halves: the first
                        # half's AG/combine/E-scale hide under phase C's tail
                        NH = NG // 2
                        Mneg = lp.tile([P, NG], f32)
                        Ssum = lp.tile([P, NG], f32)
                        tmp = lp.tile([P, NG], f32)
                        diff = lp.tile([P, NG], f32)
                        alpha = lp.tile([P, NG], f32)
                        rec = lp.tile([P, NG], f32)
                        gath = [lp.tile([P, NCORES, 2, NH], f32, name=f"gath{h}")
                                for h in range(2)]
                        for h in range(2):
                            hs = slice(h * NH, (h + 1) * NH)
                            nc.sync.dma_start(out=cc_st_in[h][:, 0:NH],
                                              in_=stats[:, 0, hs])
                            nc.sync.dma_start(out=cc_st_in[h][:, NH:NG],
                                              in_=stats[:, 1, hs])
                            collective("AllGather", ALU.bypass,
                                       [cc_st_in[h]], [cc_st_out[h]])
                            nc.sync.dma_start(
                                out=gath[h].rearrange("p c a b -> p (c a b)"),
                                in_=cc_st_out[h].rearrange("c p x -> p c x"))
                            g_h = gath[h]
                            nc.vector.tensor_copy(out=Mneg[:, hs],
                                                  in_=g_h[:, 0, 0, :])
                            for c in range(1, NCORES):
                                nc.vector.tensor_tensor(out=Mneg[:, hs],
                                                        in0=Mneg[:, hs],
                                                        in1=g_h[:, c, 0, :],
                                                        op=ALU.min)
                            for c in range(NCORES):
                                nc.vector.tensor_sub(out=tmp[:, hs],
                                                     in0=Mneg[:, hs],
                                                     in1=g_h[:, c, 0, :])
                                nc.scalar.activation(out=tmp[:, hs],
                                                     in_=tmp[:, hs], func=AF.Exp)
                                nc.vector.tensor_mul(out=tmp[:, hs],
                                                     in0=tmp[:, hs],
                                                     in1=g_h[:, c, 1, :])
                                if c == 0:
                                    nc.vector.tensor_copy(out=Ssum[:, hs],
                                                          in_=tmp[:, hs])
                                else:
                                    nc.vector.tensor_add(out=Ssum[:, hs],
                                                         in0=Ssum[:, hs],
                                                         in1=tmp[:, hs])
                            # f = exp(Mneg - mneg_local) / Ssum, fold into E
                            nc.vector.tensor_sub(out=diff[:, hs], in0=Mneg[:, hs],
                                                 in1=stats[:, 0, hs])
                            nc.scalar.activation(out=alpha[:, hs], in_=diff[:, hs],
                                                 func=AF.Exp)
                            nc.vector.reciprocal(out=rec[:, hs], in_=Ssum[:, hs])
                            nc.vector.tensor_mul(out=f_sc[:, hs], in0=alpha[:, hs],
                                                 in1=rec[:, hs])
                            for g in range(h * NH, (h + 1) * NH):
                                nc.scalar.activation(
                                    out=E[:, g, :], in_=E[:, g, :], func=AF.Copy,
                                    scale=f_sc[:, g:g + 1])

                    if phases == "AC":
                        with tc.tile_pool(name="probe", bufs=2) as prb:
                            pe_t = prb.tile([P, NL], f32, tag="pe")
                            nc.vector.tensor_copy(out=pe_t, in_=E[:, NG - 1, :])
                            nc.sync.dma_start(out=out_h.ap()[0:P, :], in_=pe_t)
                            nc.sync.dma_start(out=out_h.ap()[P:2 * P, 0:NG],
                                              in_=f_sc)
                        ep_cm.__exit__(None, None, None)
                        continue

                    # -------- phase E: out = E^T.T @ V + p (two i-half passes) ----
                    with (
                        tc.tile_pool(name="prp", bufs=2) as prp,
                        tc.tile_pool(name="osp", bufs=2) as osp,
                        tc.tile_pool(name="psE", bufs=1, space="PSUM") as psE,
                    ):
                        for ihalf in range(2):
                            po = [psE.tile([P, D], f32, tag=f"po{q_}",
                                           name=f"po{q_}")
                                  for q_ in range(IB // 2)]
                            for g in range(NG):
                                c_idx, jlb = divmod(g, JBL)
                                if ihalf == 0 and g < 4:
                                    vt = vt_pre[g]
                                else:
                                    vt = vtp.tile([P, D], f16, tag="vt")
                                    nc.gpsimd.dma_start(
                                        out=vt,
                                        in_=cc_v_out[c_idx,
                                                     jlb * P:(jlb + 1) * P, :])
                                for q_ in range(IB // 2):
                                    ib = ihalf * (IB // 2) + q_
                                    for dvh in range(2):
                                        nc.tensor.matmul(
                                            po[q_][:, dvh * FH:(dvh + 1) * FH],
                                            lhsT=E[:, g, ib * P:(ib + 1) * P],
                                            rhs=vt[:, dvh * FH:(dvh + 1) * FH],
                                            start=(g == 0), stop=(g == NG - 1))
                            for q_ in range(IB // 2):
                                ib = ihalf * (IB // 2) + q_
                                pr = prp.tile([P, D], f32, tag="pr")
                                nc.gpsimd.dma_start(
                                    out=pr, in_=pres_h.ap()[ib * P:(ib + 1) * P, :])
                                ot = osp.tile([P, D], f32, tag="ot")
                                nc.vector.tensor_add(out=ot, in0=po[q_], in1=pr)
                                nc.sync.dma_start(
                                    out=out_h.ap()[ib * P:(ib + 1) * P, :], in_=ot)
                    vtp_cm.__exit__(None, None, None)
                    ep_cm.__exit__(None, None, None)
            if spin_us:
                with tc.tile_critical():
                    for _ in range(spin_us):
                        nc.vector.nop(cycle_cnt=960)
    nc.compile()
    return nc


def prepare_in_maps(p, r, Wh, bh, Wl, bl, Wg, bg):
    f16 = np.float16
    f32 = np.float32
    WhT = np.ascontiguousarray(Wh.T).astype(f16)
    WlT = np.ascontiguousarray(Wl.T).astype(f16)
    WgT = np.ascontiguousarray(Wg.T).astype(f16)
    bl_r = np.ascontiguousarray(bl.astype(f32).reshape(DB, P).T)
    bg16 = bg.astype(f16).reshape(1, D)
    in_maps = []
    for c in range(NCORES):
        sl = slice(c * NL, (c + 1) * NL)
        in_maps.append({
            "pT": np.ascontiguousarray(p[sl].T).astype(f16),
            "rT": np.ascontiguousarray(r[sl].T).astype(f16),
            "pres": np.ascontiguousarray(p[sl]).astype(f32),
            "WhT": WhT, "WlT": WlT, "WgT": WgT,
            "bl_r": bl_r, "bg16": bg16, "ones16": np.ones((1, P), f16),
        })
    return in_maps


_NC_CACHE = {}


def kernel(p, r, Wh, bh, Wl, bl, Wg, bg):
    from concourse.bass_utils import run_bass_kernel_spmd

    p = np.asarray(p); r = np.asarray(r)
    in_maps = prepare_in_maps(p, r, np.asarray(Wh), np.asarray(bh),
                              np.asarray(Wl), np.asarray(bl),
                              np.asarray(Wg), np.asarray(bg))
    if 1 not in _NC_CACHE:
        _NC_CACHE[1] = build_nc(1)
    res = run_bass_kernel_spmd(_NC_CACHE[1], in_maps, list(range(NCORES)))
    out = np.concatenate([res.results[c]["out"] for c in range(NCORES)], axis=0)
    return out.astype(np.float32)



# revision 14
# speedup vs baseline: 3.7910x; 3.7910x over previous
"""Trainium2 Bass kernel for nn_Attention_45303315038988.

  q = p @ Wh.T (+bh) ; k = r @ Wl.T + bl ; v = p @ Wg.T + bg     [N, D]
  scores = q @ k.T ; attn = softmax(scores, axis=0) ; out = p + attn @ v

Design (8 NeuronCores, sequence-parallel over the query/row axis):
  - Weight fusion: scores = q k^T = p (Wh^T Wl) r^T + (p Wh^T bl) 1^T
    + [per-key terms that softmax over the query axis cancels].
    Host precomputes M = Wh^T Wl (fp16) and pc = p_shard (Wh^T bl); the k
    projection and its AllGather disappear entirely — phase C contracts the
    raw r^T (full, feature-major fp16, streamed from local HBM) against
    q'^T = M^T p^T. pc is added into the scores PSUM via a one-time
    partition-replicated row tile (built by a ones outer-product matmul).
  - Each core computes its v shard ([j, d], bias via a K=1 ones-row matmul),
    AllGathered in fp16 right after phase A's v projection.
  - scores^T = rT.T @ q'^T puts the softmax axis on the free dim: per-key
    max is a DVE reduction, E = exp(s - m_local) goes fp16-resident in SBUF
    (16 MB), per-key sums are DVE reductions over E.
  - Softmax globalization: (max,sum) stats are AllGathered in two halves
    (the first hides under phase C) and combined locally; the correction
    exp(m_local - M)/S is folded into E rows by ACT per-partition scales.
  - out = E^T.T @ V over 64 key blocks in two i-half passes (4 PSUM
    accumulators x 2 banks each); V streams fp16 with contiguous 2 KB rows;
    residual p tiles are prefetched at phase-E start; residual add in fp32.
  - Queue split: collectives live alone on gpsimd (plus phase-A loads and
    late pres prefetch); bulk rT/V streaming rides sync/HWDGE so it is
    never ordered behind a collective.
All matmul operands are fp16 with fp32 PSUM accumulation; softmax
statistics are fp32.
"""
import numpy as np

P = 128
D = 1024
N = 8192
NCORES = 8
NL = N // NCORES
DB = D // P
JBL = NL // P
NG = N // P
IB = NL // P
FH = 512


def build_nc(k_iters: int = 1, no_cc: bool = False, phases: str = "full",
             opts: dict | None = None, spin_us: int = 0):
    opts = opts or {}
    import concourse.mybir as mybir
    import concourse.tile as tile
    from concourse import bacc

    f16 = mybir.dt.float16
    f32 = mybir.dt.float32
    AF = mybir.ActivationFunctionType
    AX = mybir.AxisListType
    ALU = mybir.AluOpType
    RG = [list(range(NCORES))]

    nc = bacc.Bacc("TRN2", target_bir_lowering=False, debug=False,
                   num_devices=1 if no_cc else NCORES)

    def collective(kind, op, ins, outs):
        if no_cc:
            src_ap, dst_ap = ins[0], outs[0]
            nc.sync.dma_start(out=dst_ap[0] if kind == "AllGather" else dst_ap[:],
                              in_=src_ap[:])
        else:
            nc.gpsimd.collective_compute(kind, op, replica_groups=RG,
                                         ins=[ins[0].opt()], outs=[outs[0].opt()])

    pT_h = nc.dram_tensor("pT", [D, NL], f16, kind="ExternalInput")
    rT_h = nc.dram_tensor("rTf", [D, N], f16, kind="ExternalInput")
    pres_h = nc.dram_tensor("pres", [NL, D], f32, kind="ExternalInput")
    M_h = nc.dram_tensor("M16", [D, D], f16, kind="ExternalInput")
    WgT_h = nc.dram_tensor("WgT", [D, D], f16, kind="ExternalInput")
    pc_h = nc.dram_tensor("pc16", [1, NL], f16, kind="ExternalInput")
    bg_h = nc.dram_tensor("bg16", [1, D], f16, kind="ExternalInput")
    ones_h = nc.dram_tensor("ones16", [1, P], f16, kind="ExternalInput")
    out_h = nc.dram_tensor("out", [NL, D], f32, kind="ExternalOutput")

    with tile.TileContext(nc) as tc:
        with tc.tile_pool(name="dram", bufs=1, space="DRAM") as dpool:
            for it in range(k_iters):
                cc_v_in = dpool.tile([NL, D], f16, name=f"cc_v_in{it}")
                cc_v_out = dpool.tile([NCORES, NL, D], f16,
                                      addr_space="Shared", name=f"cc_v_out{it}")
                cc_st_in = [dpool.tile([P, NG], f32, name=f"cc_st_in{it}_{h}")
                            for h in range(2)]
                cc_st_out = [dpool.tile([NCORES, P, NG], f32, addr_space="Shared",
                                        name=f"cc_st_out{it}_{h}")
                             for h in range(2)]

                with tc.tile_pool(name="lp", bufs=1) as lp:
                    qT = lp.tile([P, DB, NL], f16)
                    stats = lp.tile([P, 2, NG], f32)   # [:,0,:]=-max, [:,1,:]=sum
                    f_sc = lp.tile([P, NG], f32)
                    pcrep = lp.tile([P, NL], f32)
                    bgrep = lp.tile([P, D], f16)
                    pc_sb = lp.tile([1, NL], f16)
                    bg_sb = lp.tile([1, D], f16)
                    ones_sb = lp.tile([1, P], f16)
                    nc.sync.dma_start(out=ones_sb, in_=ones_h.ap())
                    nc.sync.dma_start(out=pc_sb, in_=pc_h.ap())
                    nc.sync.dma_start(out=bg_sb, in_=bg_h.ap())

                    # ---------------- phase A: projections ----------------
                    with (
                        tc.tile_pool(name="pw", bufs=1) as pw,
                        tc.tile_pool(name="pst", bufs=3) as pst,
                        tc.tile_pool(name="psA", bufs=2, space="PSUM") as psA,
                    ):
                        M_sb = pw.tile([P, DB, D], f16)
                        WgT_sb = pw.tile([P, DB, D], f16)
                        pT_sb = pw.tile([P, DB, NL], f16)
                        # first v-projection operands race the queue startup:
                        # pT[0] rides sync (starts earliest), WgT[0] gpsimd
                        nc.sync.dma_start(out=pT_sb[:, 0, :],
                                          in_=pT_h.ap()[0:P, :])
                        nc.gpsimd.dma_start(out=WgT_sb[:, 0, :],
                                            in_=WgT_h.ap()[0:P, :])
                        for db in range(1, DB):
                            for t_sb, t_h in ((pT_sb, pT_h), (WgT_sb, WgT_h)):
                                nc.gpsimd.dma_start(
                                    out=t_sb[:, db, :],
                                    in_=t_h.ap()[db * P:(db + 1) * P, :])
                        for db in range(DB):
                            nc.gpsimd.dma_start(
                                out=M_sb[:, db, :],
                                in_=M_h.ap()[db * P:(db + 1) * P, :])

                        # pcrep = ones^T pc, bgrep = ones^T bg : one-time
                        ps_t = psA.tile([P, NL], f32)
                        for ih in range(2):
                            nc.tensor.matmul(
                                ps_t[:, ih * FH:(ih + 1) * FH],
                                lhsT=ones_sb[:, :],
                                rhs=pc_sb[:, ih * FH:(ih + 1) * FH],
                                start=True, stop=True)
                        nc.scalar.activation(out=pcrep, in_=ps_t, func=AF.Copy)
                        ps_t = psA.tile([P, NL], f32)
                        for ih in range(2):
                            nc.tensor.matmul(
                                ps_t[:, ih * FH:(ih + 1) * FH],
                                lhsT=ones_sb[:, :],
                                rhs=bg_sb[:, ih * FH:(ih + 1) * FH],
                                start=True, stop=True)
                        nc.scalar.activation(out=bgrep, in_=ps_t, func=AF.Copy)

                        # v shard = p^T.T @ Wg^T -> [j_l, dv]; +bg via DVE add
                        for jb in range(JBL):
                            ps_t = psA.tile([P, NL], f32)
                            for db in range(DB):
                                for dvh in range(2):
                                    nc.tensor.matmul(
                                        ps_t[:, dvh * FH:(dvh + 1) * FH],
                                        lhsT=pT_sb[:, db, jb * P:(jb + 1) * P],
                                        rhs=WgT_sb[:, db, dvh * FH:(dvh + 1) * FH],
                                        start=(db == 0), stop=(db == DB - 1))
                            st = pst.tile([P, NL], f16, tag="st")
                            nc.vector.tensor_add(out=st, in0=ps_t, in1=bgrep)
                            nc.gpsimd.dma_start(out=cc_v_in[jb * P:(jb + 1) * P, :],
                                                in_=st)
                        collective("AllGather", ALU.bypass, [cc_v_in], [cc_v_out])

                        # q'^T = M.T @ p^T -> [do, i]; stays in SBUF
                        for dob in range(DB):
                            ps_t = psA.tile([P, NL], f32)
                            for db in range(DB):
                                for ih in range(2):
                                    nc.tensor.matmul(
                                        ps_t[:, ih * FH:(ih + 1) * FH],
                                        lhsT=M_sb[:, db, dob * P:(dob + 1) * P],
                                        rhs=pT_sb[:, db, ih * FH:(ih + 1) * FH],
                                        start=(db == 0), stop=(db == DB - 1))
                            nc.scalar.activation(out=qT[:, dob, :], in_=ps_t,
                                                 func=AF.Copy)

                    # -------- phase C: scores^T + local stats --------
                    # rT block loads are split across the sync and gpsimd
                    # queues (a single HWDGE ring tops out near the demand
                    # rate); the half-0 stats section is injected right after
                    # block 4's loads so its AllGather slots into the gpsimd
                    # queue between load groups and the E-rescale of the first
                    # half hides under phase C's second half.
                    ep_cm = tc.tile_pool(name="ep", bufs=1)
                    ep = ep_cm.__enter__()
                    E = ep.tile([P, NG, NL], f16)
                    NH = NG // 2
                    Mneg = lp.tile([P, NG], f32)
                    Ssum = lp.tile([P, NG], f32)
                    tmp = lp.tile([P, NG], f32)
                    diff = lp.tile([P, NG], f32)
                    alpha = lp.tile([P, NG], f32)
                    rec = lp.tile([P, NG], f32)
                    gath = [lp.tile([P, NCORES, 2, NH], f32, name=f"gath{h}")
                            for h in range(2)]

                    def stats_comm(h):
                        # stores + AllGather + combine; E-rescale is emitted
                        # separately (engine streams execute in program order,
                        # so rescale ops must be woven in by hand)
                        hs = slice(h * NH, (h + 1) * NH)
                        nc.gpsimd.dma_start(out=cc_st_in[h][:, 0:NH],
                                            in_=stats[:, 0, hs])
                        nc.gpsimd.dma_start(out=cc_st_in[h][:, NH:NG],
                                            in_=stats[:, 1, hs])
                        collective("AllGather", ALU.bypass,
                                   [cc_st_in[h]], [cc_st_out[h]])
                        nc.gpsimd.dma_start(
                            out=gath[h].rearrange("p c a b -> p (c a b)"),
                            in_=cc_st_out[h].rearrange("c p x -> p c x"))
                        g_h = gath[h]
                        nc.vector.tensor_copy(out=Mneg[:, hs], in_=g_h[:, 0, 0, :])
                        for c in range(1, NCORES):
                            nc.vector.tensor_tensor(out=Mneg[:, hs],
                                                    in0=Mneg[:, hs],
                                                    in1=g_h[:, c, 0, :],
                                                    op=ALU.min)
                        for c in range(NCORES):
                            nc.vector.tensor_sub(out=tmp[:, hs],
                                                 in0=Mneg[:, hs],
                                                 in1=g_h[:, c, 0, :])
                            nc.scalar.activation(out=tmp[:, hs],
                                                 in_=tmp[:, hs], func=AF.Exp)
                            nc.vector.tensor_mul(out=tmp[:, hs],
                                                 in0=tmp[:, hs],
                                                 in1=g_h[:, c, 1, :])
                            if c == 0:
                                nc.vector.tensor_copy(out=Ssum[:, hs],
                                                      in_=tmp[:, hs])
                            else:
                                nc.vector.tensor_add(out=Ssum[:, hs],
                                                     in0=Ssum[:, hs],
                                                     in1=tmp[:, hs])
                        # f = exp(Mneg - mneg_local) / Ssum, fold into E
                        nc.vector.tensor_sub(out=diff[:, hs], in0=Mneg[:, hs],
                                             in1=stats[:, 0, hs])
                        nc.scalar.activation(out=alpha[:, hs], in_=diff[:, hs],
                                             func=AF.Exp)
                        nc.vector.reciprocal(out=rec[:, hs], in_=Ssum[:, hs])
                        nc.vector.tensor_mul(out=f_sc[:, hs], in0=alpha[:, hs],
                                             in1=rec[:, hs])

                    def rescale(g, eng):
                        if eng is nc.scalar:
                            nc.scalar.activation(
                                out=E[:, g, :], in_=E[:, g, :], func=AF.Copy,
                                scale=f_sc[:, g:g + 1])
                        else:
                            eng.tensor_scalar_mul(out=E[:, g, :],
                                                  in0=E[:, g, :],
                                                  scalar1=f_sc[:, g:g + 1])

                    with (
                        tc.tile_pool(name="ktp", bufs=2) as ktp,
                        tc.tile_pool(name="psC", bufs=3, space="PSUM") as psC,
                    ):
                        rt_tiles = {}

                        def rt_load(blk):
                            rt_c = ktp.tile([P, DB, JBL, P], f16, tag="kt")
                            for db in range(DB):
                                eng = nc.sync if db < DB // 2 else nc.gpsimd
                                eng.dma_start(
                                    out=rt_c[:, db, :, :].rearrange(
                                        "p a b -> p (a b)"),
                                    in_=rT_h.ap()[db * P:(db + 1) * P,
                                                  blk * NL:(blk + 1) * NL])
                            rt_tiles[blk] = rt_c

                        rescale_fifo = []

                        def rt_groups(blk):
                            rt_c = rt_tiles.pop(blk)
                            for jlb in range(JBL):
                                g = blk * JBL + jlb
                                ps_t = psC.tile([P, NL], f32, tag="sc")
                                for db in range(DB):
                                    for ih in range(2):
                                        nc.tensor.matmul(
                                            ps_t[:, ih * FH:(ih + 1) * FH],
                                            lhsT=rt_c[:, db, jlb, :],
                                            rhs=qT[:, db, ih * FH:(ih + 1) * FH],
                                            start=(db == 0), stop=(db == DB - 1))
                                nc.vector.tensor_add(out=ps_t, in0=ps_t,
                                                     in1=pcrep)
                                nc.vector.tensor_reduce(
                                    out=stats[:, 0, g:g + 1], in_=ps_t,
                                    op=ALU.max, axis=AX.X, negate=True)
                                nc.scalar.activation(
                                    out=E[:, g, :], in_=ps_t, func=AF.Exp,
                                    bias=stats[:, 0, g:g + 1], scale=1.0,
                                    accum_out=stats[:, 1, g:g + 1])
                                if rescale_fifo:
                                    rescale(rescale_fifo.pop(0), nc.scalar)

                        for blk in range(4):
                            rt_load(blk)
                            rt_groups(blk)
                        rt_load(4)
                        stats_comm(0)
                        rt_groups(4)
                        rt_load(5)
                        rt_groups(5)
                        # weave one h0-rescale per group into blocks 6-7 so
                        # neither the ACT nor the DVE stream gets a dense
                        # rescale burst blocking phase-C ops behind it
                        rescale_fifo.extend(range(0, 16))
                        rt_load(6)
                        rt_groups(6)
                        rt_load(7)
                        rt_groups(7)

                    # prefetch the first phase-E V tiles on gpsimd before the
                    # half-1 stats collective blocks that queue
                    vtp_cm = tc.tile_pool(name="vtp", bufs=8)
                    vtp = vtp_cm.__enter__()
                    vt_pre = []
                    for g in range(8):
                        c_idx, jlb = divmod(g, JBL)
                        vt = vtp.tile([P, D], f16, tag="vt", name=f"vtpre{g}")
                        nc.gpsimd.dma_start(
                            out=vt, in_=cc_v_out[c_idx, jlb * P:(jlb + 1) * P, :])
                        vt_pre.append(vt)
                    stats_comm(1)
                    # remaining rescales: h0 leftovers first (deps satisfied,
                    # run at phase-E start), then h1 ascending; alternate
                    # ACT/DVE so the two streams drain in parallel
                    for g in range(16, NG):
                        rescale(g, nc.scalar if g % 2 == 0 else nc.vector)

                    # -------- phase E: out = E^T.T @ V + p (two i-half passes) --
                    with (
                        tc.tile_pool(name="prp", bufs=1) as prp,
                        tc.tile_pool(name="osp", bufs=2) as osp,
                        tc.tile_pool(name="psE", bufs=1, space="PSUM") as psE,
                    ):
                        for ihalf in range(2):
                            # prefetch this half's residual p tiles early so
                            # the output tail is just add+store
                            pr_tiles = []
                            for q_ in range(IB // 2):
                                ib = ihalf * (IB // 2) + q_
                                pr = prp.tile([P, D], f32, tag=f"pr{q_}")
                                nc.gpsimd.dma_start(
                                    out=pr, in_=pres_h.ap()[ib * P:(ib + 1) * P, :])
                                pr_tiles.append(pr)
                            po = [psE.tile([P, D], f32, tag=f"po{q_}",
                                           name=f"po{q_}")
                                  for q_ in range(IB // 2)]
                            for g in range(NG):
                                c_idx, jlb = divmod(g, JBL)
                                if ihalf == 0 and g < 8:
                                    vt = vt_pre[g]
                                else:
                                    vt = vtp.tile([P, D], f16, tag="vt")
                                    # keep early tiles off gpsimd (AG(st1)
                                    # still blocks that queue at phase-E start)
                                    eng = (nc.sync if (ihalf == 0 and g < 16)
                                           else (nc.sync if g % 2 else nc.gpsimd))
                                    eng.dma_start(
                                        out=vt,
                                        in_=cc_v_out[c_idx,
                                                     jlb * P:(jlb + 1) * P, :])
                                for q_ in range(IB // 2):
                                    ib = ihalf * (IB // 2) + q_
                                    for dvh in range(2):
                                        nc.tensor.matmul(
                                            po[q_][:, dvh * FH:(dvh + 1) * FH],
                                            lhsT=E[:, g, ib * P:(ib + 1) * P],
                                            rhs=vt[:, dvh * FH:(dvh + 1) * FH],
                                            start=(g == 0), stop=(g == NG - 1))
                            for q_ in range(IB // 2):
                                ib = ihalf * (IB // 2) + q_
                                for dvh in range(2):
                                    cs = slice(dvh * FH, (dvh + 1) * FH)
                                    ot = osp.tile([P, FH], f32, tag=f"ot{dvh}")
                                    nc.vector.tensor_add(out=ot,
                                                         in0=po[q_][:, cs],
                                                         in1=pr_tiles[q_][:, cs])
                                    eng = nc.sync if (q_ + dvh) % 2 else nc.gpsimd
                                    eng.dma_start(
                                        out=out_h.ap()[ib * P:(ib + 1) * P, cs],
                                        in_=ot)
                    vtp_cm.__exit__(None, None, None)
                    ep_cm.__exit__(None, None, None)
            if spin_us:
                with tc.tile_critical():
                    for _ in range(spin_us):
                        nc.vector.nop(cycle_cnt=960)
    nc.compile()
    return nc


def prepare_in_maps(p, r, Wh, bh, Wl, bl, Wg, bg):
    f16 = np.float16
    f32 = np.float32
    p = np.asarray(p, dtype=f32)
    r = np.asarray(r, dtype=f32)
    Wh64 = np.asarray(Wh, dtype=np.float64)
    Wl64 = np.asarray(Wl, dtype=np.float64)
    M16 = np.ascontiguousarray(Wh64.T @ Wl64).astype(f16)
    c32 = (Wh64.T @ np.asarray(bl, dtype=np.float64)).astype(f32)
    WgT = np.ascontiguousarray(np.asarray(Wg).T).astype(f16)
    bg16 = np.asarray(bg).astype(f16).reshape(1, D)
    rTf = np.ascontiguousarray(r.T).astype(f16)
    in_maps = []
    for c in range(NCORES):
        sl = slice(c * NL, (c + 1) * NL)
        pc16 = (p[sl] @ c32).astype(f16).reshape(1, NL)
        in_maps.append({
            "pT": np.ascontiguousarray(p[sl].T).astype(f16),
            "rTf": rTf,
            "pres": np.ascontiguousarray(p[sl]).astype(f32),
            "M16": M16, "WgT": WgT, "pc16": pc16,
            "bg16": bg16, "ones16": np.ones((1, P), f16),
        })
    return in_maps


_NC_CACHE = {}


def kernel(p, r, Wh, bh, Wl, bl, Wg, bg):
    from concourse.bass_utils import run_bass_kernel_spmd

    p = np.asarray(p); r = np.asarray(r)
    in_maps = prepare_in_maps(p, r, np.asarray(Wh), np.asarray(bh),
                              np.asarray(Wl), np.asarray(bl),
                              np.asarray(Wg), np.asarray(bg))
    if 1 not in _NC_CACHE:
        _NC_CACHE[1] = build_nc(1)
    res = run_bass_kernel_spmd(_NC_CACHE[1], in_maps, list(range(NCORES)))
    out = np.concatenate([res.results[c]["out"] for c in range(NCORES)], axis=0)
    return out.astype(np.float32)
